# revision 9
# baseline (speedup 1.0000x reference)
"""DGMNet (dense MLP, 4 DGM layers) Trainium2 kernel.

Strategy: data-parallel over the batch dim (65536 rows -> 8 cores x 8192).
Inside each core, activations live feature-major in SBUF ([128 features x
batch-tile] tiles); every gate matmul is out[M=feat,N=batch] =
W.T-slice @ S with PE accumulation over the 1024-feature contraction, and
the x-side projections (K=16) are folded into the same PSUM accumulation
group, issued 4-wide via row tiling.

Algebraic reduction: S1 = x@Sw.T + b is affine in the 16-dim input, so
every layer-0 term that is linear in S1 collapses into a K=16 matmul with
host-folded weights:
    Wz@S1 -> x@(Wz Sw).T + Wz Sw_b      (layer-0 Z gate)
    Wg@S1 -> x@(Wg Sw).T + Wg Sw_b      (G gate and layer-0 R gate)
This removes 2 of the 12 HxH matmuls per batch element (and the whole
wgS1 tensor + its DVE adds/ACT evacuations).

Matmuls run in bfloat16 (fp8 was measured to blow the 2e-2 error budget:
weight-only e4m3 already gives 2.4e-2). PSUM accumulation stays fp32;
biases ride the ACT evacuation; the elementwise combine runs on the DVE
in bf16 via scalar_tensor_tensor ((G-1)*H) so no (1-G) precompute is
needed. The scalar output row of tile t is emitted during tile t+1's
phase-A so the PE never waits on the layer-3 tail.
"""

import sys

sys.path.insert(0, "/opt/trn_rl_repo")

import numpy as np

B_FULL = 65536
KI = 16
H = 1024
NCORES = 8
BC = B_FULL // NCORES  # per-core batch (8192)
NB = 512               # batch tile (one PSUM bank of fp32)
NM = H // 128          # feature tiles (8)
N_LAYERS = 4
NG = 7                 # U gate blocks: S1, Z0', G', R0', Z, R, H

MM_DT = "bfloat16"

_BUILD_CACHE = {}


def _build(bc, nb, mm_dt=MM_DT, repeat=1):
    """Build + compile the single-core Bass program. Returns nc.

    repeat > 1 re-runs the whole computation (for slope-based timing of the
    device execution under the large axon dispatch overhead)."""
    import concourse.bacc as bacc
    import concourse.mybir as mybir
    import concourse.tile as tile

    f32 = mybir.dt.float32
    mdt = getattr(mybir.dt, mm_dt)
    Tanh = mybir.ActivationFunctionType.Tanh
    mult = mybir.AluOpType.mult
    sub = mybir.AluOpType.subtract

    nt = bc // nb

    nc = bacc.Bacc("TRN2", target_bir_lowering=False, debug=False,
                   num_devices=NCORES)

    xT_d = nc.dram_tensor("xT", [KI, bc], mdt, kind="ExternalInput").ap()
    wz_d = nc.dram_tensor("WzT", [H, H], mdt, kind="ExternalInput").ap()
    wg_d = nc.dram_tensor("WgT", [H, H], mdt, kind="ExternalInput").ap()
    u_d = nc.dram_tensor("U", [128, NG * H], mdt, kind="ExternalInput").ap()
    bias_d = nc.dram_tensor("BIAS", [128, 64], f32, kind="ExternalInput").ap()
    ow_d = nc.dram_tensor("OW", [128, NM], mdt, kind="ExternalInput").ap()
    y_d = nc.dram_tensor("Y", [1, bc], f32, kind="ExternalOutput").ap()

    with tile.TileContext(nc) as tc:
        with (
            tc.tile_pool(name="const", bufs=1) as cpool,
            tc.tile_pool(name="xt", bufs=3) as xt_pool,
            tc.tile_pool(name="s", bufs=3) as s_pool,
            tc.tile_pool(name="act", bufs=2) as act_pool,
            tc.tile_pool(name="ov", bufs=2) as ov_pool,
            tc.tile_pool(name="psum", bufs=7, space="PSUM") as ps_pool,
            tc.tile_pool(name="pso", bufs=1, space="PSUM") as pso_pool,
        ):
            # ---- resident constants. Order matters: the first tile's
            # phase A needs only U + bias + x, so those go ahead of the
            # 4 MB of Wg/Wz; Wg is needed a layer-phase before Wz. ---------
            u_sb = cpool.tile([128, NG * H], mdt)
            nc.gpsimd.dma_start(u_sb[:], u_d[:])
            bias_sb = cpool.tile([128, 64], f32)
            nc.gpsimd.dma_start(bias_sb[:], bias_d[:])
            wg_sb = cpool.tile([128, NM * H], mdt)
            wz_sb = cpool.tile([128, NM * H], mdt)
            ow_sb = cpool.tile([128, NM], mdt)

            def load_big_weights():
                for k in range(NM):
                    nc.gpsimd.dma_start(wg_sb[:, k * H:(k + 1) * H],
                                        wg_d[k * 128:(k + 1) * 128, :])
                nc.gpsimd.dma_start(ow_sb[:], ow_d[:])
                for k in range(NM):
                    nc.gpsimd.dma_start(wz_sb[:, k * H:(k + 1) * H],
                                        wz_d[k * 128:(k + 1) * 128, :])

            def w_ap(w_sb, k, m):
                return w_sb[:, k * H + m * 128:k * H + (m + 1) * 128]

            def u_ap(g, m, c):
                return u_sb[32 * c:32 * c + KI,
                            g * H + m * 128:g * H + (m + 1) * 128]

            def b_ap(g, m):
                return bias_sb[:, g * NM + m:g * NM + m + 1]

            def x_starts(gate, xt, nametag, single):
                """Row-tiled (4-concurrent) K=16 start matmuls for m-quads."""
                pss = [None] * NM
                for mq in (0, 4):
                    for c in range(4):
                        m = mq + c
                        pss[m] = ps_pool.tile([128, nb], f32, tag="ps",
                                              name=f"{nametag}_{m}")
                    for c in range(4):
                        m = mq + c
                        nc.tensor.matmul(
                            pss[m][:], u_ap(gate, m, c),
                            xt[32 * c:32 * c + KI, :],
                            start=True, stop=single,
                            tile_position=(32 * c, 0))
                return pss

            def wgroup(gate, w_sb, xt, rhs, dest, t_u, li, tag):
                """Full gate: U-start quad + 8 K-chunk matmuls per m, tanh
                evacuation with bias into dest[m]."""
                for mq in (0, 4):
                    pss = {}
                    for c in range(4):
                        m = mq + c
                        pss[m] = ps_pool.tile([128, nb], f32, tag="ps",
                                              name=f"ps_{tag}_{t_u}_{li}_{m}")
                    for c in range(4):
                        m = mq + c
                        nc.tensor.matmul(pss[m][:], u_ap(gate, m, c),
                                         xt[32 * c:32 * c + KI, :],
                                         start=True, stop=False,
                                         tile_position=(32 * c, 0))
                    for c in range(4):
                        m = mq + c
                        for k in range(NM):
                            nc.tensor.matmul(pss[m][:], w_ap(w_sb, k, m),
                                             rhs[k][:],
                                             start=False, stop=(k == NM - 1))
                        nc.scalar.activation(dest[m][:], pss[m][:], Tanh,
                                             bias=b_ap(gate, m))

            # ---- per batch tile -----------------------------------------
            pend = None  # deferred output row of the previous batch tile

            def emit_out(pend):
                h_prev, tp, up = pend
                po = pso_pool.tile([1, nb], f32, tag="po", name=f"po_{up}")
                for k in range(NM):
                    nc.tensor.matmul(po[:], ow_sb[:, k:k + 1], h_prev[k][:],
                                     start=(k == 0), stop=(k == NM - 1))
                orow = ov_pool.tile([1, nb], f32, tag="orow", name=f"orow_{up}")
                nc.vector.tensor_scalar_add(orow[:], po[:],
                                            bias_sb[0:1, 56:57])
                nc.gpsimd.dma_start(y_d[0:1, tp * nb:(tp + 1) * nb], orow[:])

            ntot = repeat * nt

            def load_xt(u):
                t = u % nt
                xt = xt_pool.tile([128, nb], mdt, tag="xt", name=f"xt_{u}")
                for c in range(4):
                    nc.gpsimd.dma_start(xt[32 * c:32 * c + KI, :],
                                        xT_d[:, t * nb:(t + 1) * nb])
                return xt

            def phase_s1(u, xt):
                """S1 = x@Sw.T + b; evacuates on the DVE so the ACT engine
                stays free for R0/G/Z0."""
                s0 = [s_pool.tile([128, nb], mdt, tag=f"s{k}",
                                  name=f"s_{u}_0_{k}") for k in range(NM)]
                ps1 = x_starts(0, xt, f"ps_s1_{u}", single=True)
                for m in range(NM):
                    nc.vector.tensor_scalar_add(s0[m][:], ps1[m][:],
                                                b_ap(0, m))
                return s0

            def phase_gate(u, xt, gate, tag):
                dest = [act_pool.tile([128, nb], mdt, tag=f"{tag}{m}",
                                      name=f"{tag}_{u}_0_{m}")
                        for m in range(NM)]
                ps = x_starts(gate, xt, f"ps_{tag}0_{u}", single=True)
                for m in range(NM):
                    nc.scalar.activation(dest[m][:], ps[m][:], Tanh,
                                         bias=b_ap(gate, m))
                return dest

            # software-pipelined prologue: tile 0's phase A
            xt = load_xt(0)
            load_big_weights()
            state = {
                "s": phase_s1(0, xt),
                "r": phase_gate(0, xt, 3, "r"),
                "g": phase_gate(0, xt, 2, "g"),
                "z": phase_gate(0, xt, 1, "z"),
                "xt": xt,
            }

            for u in range(ntot):
                t = u % nt
                xt = state["xt"]
                s_cur, r_t, g_t, z_t = (state["s"], state["r"], state["g"],
                                        state["z"])
                xt_n = load_xt(u + 1) if u + 1 < ntot else None

                # previous tile's output row fills the SR0/H0 dep gap
                if pend is not None:
                    emit_out(pend)
                    pend = None

                for i in range(N_LAYERS):
                    if i > 0:
                        # R = tanh(br + Ur x + Wg S) — on H's critical
                        # path, so it goes before Z.
                        r_t = [act_pool.tile([128, nb], mdt, tag=f"r{m}",
                                             name=f"r_{u}_{i}_{m}")
                               for m in range(NM)]
                        wgroup(5, wg_sb, xt, s_cur, r_t, u, i, "r")
                        # Z = tanh(bz + Uz x + Wz S)
                        z_t = [act_pool.tile([128, nb], mdt, tag=f"z{m}",
                                             name=f"z_{u}_{i}_{m}")
                               for m in range(NM)]
                        wgroup(4, wz_sb, xt, s_cur, z_t, u, i, "z")

                    # SR = S * R, in place into R's tiles
                    for k in range(NM):
                        nc.vector.tensor_mul(r_t[k][:], s_cur[k][:],
                                             r_t[k][:])

                    # H = tanh(bh + Uh x + Wg (S*R))
                    h_t = [act_pool.tile([128, nb], mdt, tag=f"h{m}",
                                         name=f"h_{u}_{i}_{m}")
                           for m in range(NM)]
                    # Z*S can run on the DVE during H's matmuls
                    for m in range(NM):
                        nc.vector.tensor_mul(z_t[m][:], z_t[m][:],
                                             s_cur[m][:])
                    wgroup(6, wg_sb, xt, r_t, h_t, u, i, "h")

                    # output = (1-G)*H + Z*S == (Z*S) - (G-1)*H
                    for m in range(NM):
                        nc.vector.scalar_tensor_tensor(
                            h_t[m][:], g_t[m][:], 1.0, h_t[m][:],
                            op0=sub, op1=mult)
                        nc.vector.tensor_sub(h_t[m][:], z_t[m][:],
                                             h_t[m][:])

                    if i < N_LAYERS - 1:
                        s_new = [s_pool.tile([128, nb], mdt, tag=f"s{k}",
                                             name=f"s_{u}_{i + 1}_{k}")
                                 for k in range(NM)]
                        for m in range(NM):
                            nc.scalar.activation(s_new[m][:], h_t[m][:],
                                                 Tanh)
                        s_cur = s_new

                    # next tile's phase-A pieces fill the PE stall where
                    # layer i+1's first k-matmul waits on s_new planes.
                    # (Placement is WAR-constrained: r/z prefetches must come
                    # after tile t's last alloc of the same tag at bufs=2.)
                    if xt_n is not None:
                        if i == 0:
                            state["s"] = phase_s1(u + 1, xt_n)
                        elif i == 1:
                            state["g"] = phase_gate(u + 1, xt_n, 2, "g")
                        elif i == 2:
                            state["r"] = phase_gate(u + 1, xt_n, 3, "r")
                            state["z"] = phase_gate(u + 1, xt_n, 1, "z")

                # y = out_w @ output + out_b, deferred into the next
                # tile's H0 phase
                pend = (h_t, t, u)
                state["xt"] = xt_n

            if pend is not None:
                emit_out(pend)

    nc.compile()
    return nc


def _get_nc(bc=BC, nb=NB, mm_dt=MM_DT):
    key = (bc, nb, mm_dt)
    if key not in _BUILD_CACHE:
        _BUILD_CACHE[key] = _build(bc, nb, mm_dt)
    return _BUILD_CACHE[key]


def _prep_inputs(x, Sw_w, Sw_b, Uz_w, Uz_b, Wz_w, Wz_b, Ug_w, Ug_b, Wg_w,
                 Wg_b, Ur_w, Ur_b, Uh_w, Uh_b, out_w, out_b):
    f = np.float32
    d = np.float64
    xT = np.ascontiguousarray(np.asarray(x, f).T)               # [16, B]
    WzT = np.ascontiguousarray(np.asarray(Wz_w, f).T)           # [H, H]
    WgT = np.ascontiguousarray(np.asarray(Wg_w, f).T)

    # layer-0 folds (S1 is affine in x)
    Swd, Swbd = np.asarray(Sw_w, d), np.asarray(Sw_b, d)
    Wzd, Wgd = np.asarray(Wz_w, d), np.asarray(Wg_w, d)
    MzS = Wzd @ Swd            # [H, 16]
    MgS = Wgd @ Swd
    czS = Wzd @ Swbd           # [H]
    cgS = Wgd @ Swbd

    ublocks = [
        np.asarray(Sw_w, d),
        np.asarray(Uz_w, d) + MzS,
        np.asarray(Ug_w, d) + MgS,
        np.asarray(Ur_w, d) + MgS,
        np.asarray(Uz_w, d),
        np.asarray(Ur_w, d),
        np.asarray(Uh_w, d),
    ]
    U16 = np.concatenate([w.T for w in ublocks], axis=1).astype(f)  # [16,7H]
    U = np.zeros((128, NG * H), f)
    for c in range(4):
        U[32 * c:32 * c + KI] = U16

    combos = [
        np.asarray(Sw_b, d),
        np.asarray(Uz_b, d) + np.asarray(Wz_b, d) + czS,
        np.asarray(Ug_b, d) + np.asarray(Wg_b, d) + cgS,
        np.asarray(Ur_b, d) + np.asarray(Wg_b, d) + cgS,
        np.asarray(Uz_b, d) + np.asarray(Wz_b, d),
        np.asarray(Ur_b, d) + np.asarray(Wg_b, d),
        np.asarray(Uh_b, d) + np.asarray(Wg_b, d),
    ]
    bias = np.zeros((128, 64), f)
    for g, b in enumerate(combos):
        bias[:, g * NM:(g + 1) * NM] = b.astype(f).reshape(NM, 128).T
    bias[:, 56] = np.float32(np.asarray(out_b, f)[0])
    OW = np.ascontiguousarray(np.asarray(out_w, f).reshape(NM, 128).T)
    return xT, WzT, WgT, U, bias, OW


def kernel(**inputs):
    from concourse.bass_utils import run_bass_kernel_spmd

    nc = _get_nc()
    in_maps = _make_in_maps(inputs)
    res = run_bass_kernel_spmd(nc, in_maps, list(range(NCORES)))
    y = np.concatenate([res.results[c]["Y"] for c in range(NCORES)], axis=1)
    return np.ascontiguousarray(y.reshape(B_FULL, 1)).astype(np.float32)


def _make_in_maps(inputs, mm_dt=MM_DT):
    from concourse import mybir

    xT, WzT, WgT, U, bias, OW = _prep_inputs(**inputs)
    mnp = mybir.dt.np(getattr(mybir.dt, mm_dt))
    cast = lambda a: np.ascontiguousarray(a).astype(mnp)
    return [{
        "xT": cast(xT[:, c * BC:(c + 1) * BC]),
        "WzT": cast(WzT), "WgT": cast(WgT), "U": cast(U), "BIAS": bias,
        "OW": cast(OW),
    } for c in range(NCORES)]


def timed_run(inputs, iters=5, nc=None, pipeline=1):
    """Build a persistent jitted runner (so walrus compiles once), stage the
    inputs on-device, and time repeated executions. Returns (best_ns,
    all_ns, output)."""
    import time
    import jax
    from jax.sharding import Mesh, PartitionSpec, NamedSharding
    from jax.experimental.shard_map import shard_map
    from concourse import bass2jax, mybir

    bass2jax.install_neuronx_cc_hook()
    if nc is None:
        nc = _get_nc()
    in_maps = _make_in_maps(inputs)
    n_cores = NCORES

    partition_name = (nc.partition_id_tensor.name
                      if nc.partition_id_tensor else None)
    in_names, out_names, out_avals, zero_outs = [], [], [], []
    for alloc in nc.m.functions[0].allocations:
        if not isinstance(alloc, mybir.MemoryLocationSet):
            continue
        name = alloc.memorylocations[0].name
        if alloc.kind == "ExternalInput":
            if name != partition_name:
                in_names.append(name)
        elif alloc.kind == "ExternalOutput":
            shape = tuple(alloc.tensor_shape)
            dtype = mybir.dt.np(alloc.dtype)
            out_names.append(name)
            out_avals.append(jax.core.ShapedArray(shape, dtype))
            zero_outs.append(np.zeros(shape, dtype))
    n_params = len(in_names)
    n_outs = len(out_avals)
    all_in = list(in_names) + list(out_names)
    if partition_name is not None:
        all_in.append(partition_name)
    donate = tuple(range(n_params, n_params + n_outs))

    def _body(*args):
        operands = list(args)
        if partition_name is not None:
            operands.append(bass2jax.partition_id_tensor())
        outs = bass2jax._bass_exec_p.bind(
            *operands,
            out_avals=tuple(out_avals),
            in_names=tuple(all_in),
            out_names=tuple(out_names),
            lowering_input_output_aliases=(),
            sim_require_finite=True,
            sim_require_nnan=True,
            nc=nc,
        )
        return tuple(outs)

    devices = jax.devices()[:n_cores]
    mesh = Mesh(np.asarray(devices), ("core",))
    spec = PartitionSpec("core")
    sharded = jax.jit(
        shard_map(_body, mesh=mesh, in_specs=(spec,) * (n_params + n_outs),
                  out_specs=(spec,) * n_outs, check_rep=False),
        donate_argnums=donate, keep_unused=True)

    sharding = NamedSharding(mesh, spec)
    dev_in = [
        jax.device_put(
            np.concatenate([np.asarray(in_maps[c][n]) for c in range(n_cores)],
                           axis=0), sharding)
        for n in in_names
    ]
    def fresh_zeros():
        return [np.zeros((n_cores * z.shape[0], *z.shape[1:]), z.dtype)
                for z in zero_outs]

    # warmup (compiles)
    outs = sharded(*dev_in, *fresh_zeros())
    jax.block_until_ready(outs)

    state = {"outs": outs}

    def run_once(pipeline_n=pipeline):
        zss = [fresh_zeros() for _ in range(pipeline_n)]
        t0 = time.perf_counter()
        all_outs = [sharded(*dev_in, *zs) for zs in zss]
        jax.block_until_ready(all_outs)
        state["outs"] = all_outs[-1]
        return int((time.perf_counter() - t0) * 1e9 / pipeline_n)

    def get_y():
        y = np.asarray(state["outs"][out_names.index("Y")])  # [8, BC]
        return np.ascontiguousarray(
            y.reshape(1, B_FULL).reshape(B_FULL, 1)).astype(np.float32)

    if iters is None:
        return run_once, get_y

    times = [run_once() for _ in range(iters)]
    return min(times), times, get_y()


# revision 17
# speedup vs baseline: 1.1050x; 1.1050x over previous
"""DGMNet (dense MLP, 4 DGM layers) Trainium2 kernel.

Strategy: data-parallel over the batch dim (65536 rows -> 8 cores x 8192).
Inside each core, activations live feature-major in SBUF ([128 features x
batch-tile] tiles); every gate matmul is out[M=feat,N=batch] =
W.T-slice @ S with PE accumulation over the 1024-feature contraction, and
the x-side projections (K=16) are folded into the same PSUM accumulation
group, issued 4-wide via row tiling.

Algebraic reduction: S1 = x@Sw.T + b is affine in the 16-dim input, so
every layer-0 term that is linear in S1 collapses into a K=16 matmul with
host-folded weights:
    Wz@S1 -> x@(Wz Sw).T + Wz Sw_b      (layer-0 Z gate)
    Wg@S1 -> x@(Wg Sw).T + Wg Sw_b      (G gate and layer-0 R gate)
This removes 2 of the 12 HxH matmuls per batch element (and the whole
wgS1 tensor + its DVE adds/ACT evacuations).

Matmuls run in bfloat16 (fp8 was measured to blow the 2e-2 error budget:
weight-only e4m3 already gives 2.4e-2). PSUM accumulation stays fp32;
biases ride the ACT evacuation; the elementwise combine runs on the DVE
in bf16 via scalar_tensor_tensor ((G-1)*H) so no (1-G) precompute is
needed. The scalar output row of tile t is emitted during tile t+1's
phase-A so the PE never waits on the layer-3 tail.
"""

import sys

sys.path.insert(0, "/opt/trn_rl_repo")

import numpy as np

B_FULL = 65536
KI = 16
H = 1024
NCORES = 8
BC = B_FULL // NCORES  # per-core batch (8192)
NB = 512               # batch tile (one PSUM bank of fp32)
NM = H // 128          # feature tiles (8)
N_LAYERS = 4
NG = 7                 # U gate blocks: S1, Z0', G', R0', Z, R, H

MM_DT = "bfloat16"

_BUILD_CACHE = {}


def _build(bc, nb, mm_dt=MM_DT, repeat=1):
    """Build + compile the single-core Bass program. Returns nc.

    repeat > 1 re-runs the whole computation (for slope-based timing of the
    device execution under the large axon dispatch overhead)."""
    import concourse.bacc as bacc
    import concourse.mybir as mybir
    import concourse.tile as tile

    f32 = mybir.dt.float32
    mdt = getattr(mybir.dt, mm_dt)
    Tanh = mybir.ActivationFunctionType.Tanh
    mult = mybir.AluOpType.mult
    sub = mybir.AluOpType.subtract

    nt = bc // nb

    nc = bacc.Bacc("TRN2", target_bir_lowering=False, debug=False,
                   num_devices=NCORES)

    xT_d = nc.dram_tensor("xT", [KI, bc], mdt, kind="ExternalInput").ap()
    wz_d = nc.dram_tensor("WzT", [H, H], mdt, kind="ExternalInput").ap()
    wg_d = nc.dram_tensor("WgT", [H, H], mdt, kind="ExternalInput").ap()
    u_d = nc.dram_tensor("U", [128, NG * H], mdt, kind="ExternalInput").ap()
    bias_d = nc.dram_tensor("BIAS", [128, 64], f32, kind="ExternalInput").ap()
    ow_d = nc.dram_tensor("OW", [128, NM], f32, kind="ExternalInput").ap()
    y_d = nc.dram_tensor("Y", [1, bc], f32, kind="ExternalOutput").ap()

    with tile.TileContext(nc) as tc:
        with (
            tc.tile_pool(name="const", bufs=1) as cpool,
            tc.tile_pool(name="xt", bufs=3) as xt_pool,
            tc.tile_pool(name="s", bufs=3) as s_pool,
            tc.tile_pool(name="act", bufs=2) as act_pool,
            tc.tile_pool(name="ov", bufs=2) as ov_pool,
            tc.tile_pool(name="psum", bufs=8, space="PSUM") as ps_pool,
        ):
            # ---- resident constants. Order matters: the first tile's
            # phase A needs only U + bias + x, so those go ahead of the
            # 4 MB of Wg/Wz; Wg is needed a layer-phase before Wz. ---------
            u_sb = cpool.tile([128, NG * H], mdt)
            nc.gpsimd.dma_start(u_sb[:], u_d[:])
            bias_sb = cpool.tile([128, 64], f32)
            nc.gpsimd.dma_start(bias_sb[:], bias_d[:])
            wg_sb = cpool.tile([128, NM * H], mdt)
            wz_sb = cpool.tile([128, NM * H], mdt)
            ow_sb = cpool.tile([128, NM], f32)
            ones_sb = cpool.tile([128, 1], mdt)
            nc.gpsimd.memset(ones_sb[:], 1.0)

            def load_big_weights():
                for k in range(NM):
                    nc.gpsimd.dma_start(wg_sb[:, k * H:(k + 1) * H],
                                        wg_d[k * 128:(k + 1) * 128, :])
                nc.gpsimd.dma_start(ow_sb[:], ow_d[:])
                for k in range(NM):
                    nc.gpsimd.dma_start(wz_sb[:, k * H:(k + 1) * H],
                                        wz_d[k * 128:(k + 1) * 128, :])

            def w_ap(w_sb, k, m):
                return w_sb[:, k * H + m * 128:k * H + (m + 1) * 128]

            def u_ap(g, m, c):
                return u_sb[32 * c:32 * c + KI,
                            g * H + m * 128:g * H + (m + 1) * 128]

            def b_ap(g, m):
                return bias_sb[:, g * NM + m:g * NM + m + 1]

            def x_starts(gate, xt, nametag, single):
                """Row-tiled (4-concurrent) K=16 start matmuls for m-quads."""
                pss = [None] * NM
                for mq in (0, 4):
                    for c in range(4):
                        m = mq + c
                        pss[m] = ps_pool.tile([128, nb], f32, tag="ps",
                                              name=f"{nametag}_{m}")
                    for c in range(4):
                        m = mq + c
                        nc.tensor.matmul(
                            pss[m][:], u_ap(gate, m, c),
                            xt[32 * c:32 * c + KI, :],
                            start=True, stop=single,
                            tile_position=(32 * c, 0))
                return pss

            def wgroup(gate, w_sb, xt, rhs, dest, t_u, li, tag):
                """Full gate: U-start quad + 8 K-chunk matmuls per m, tanh
                evacuation with bias into dest[m]."""
                for mq in (0, 4):
                    pss = {}
                    for c in range(4):
                        m = mq + c
                        pss[m] = ps_pool.tile([128, nb], f32, tag="ps",
                                              name=f"ps_{tag}_{t_u}_{li}_{m}")
                    for c in range(4):
                        m = mq + c
                        nc.tensor.matmul(pss[m][:], u_ap(gate, m, c),
                                         xt[32 * c:32 * c + KI, :],
                                         start=True, stop=False,
                                         tile_position=(32 * c, 0))
                    for c in range(4):
                        m = mq + c
                        for k in range(NM):
                            nc.tensor.matmul(pss[m][:], w_ap(w_sb, k, m),
                                             rhs[k][:],
                                             start=False, stop=(k == NM - 1))
                        nc.scalar.activation(dest[m][:], pss[m][:], Tanh,
                                             bias=b_ap(gate, m))

            # ---- per batch tile -----------------------------------------
            pend = None  # deferred output row of the previous batch tile

            def emit_out(pend):
                # y-row = out_w @ h: per-partition weighted accumulation on
                # the DVE (8 fused mul-adds), then a single ones-matmul for
                # the partition reduction — 1 PE matmul instead of 8.
                h_prev, tp, up = pend
                v = ov_pool.tile([128, nb], mdt, tag="vrow", name=f"v_{up}")
                nc.vector.tensor_scalar_mul(v[:], h_prev[0][:],
                                            ow_sb[:, 0:1])
                for k in range(1, NM):
                    nc.vector.scalar_tensor_tensor(
                        v[:], h_prev[k][:], ow_sb[:, k:k + 1], v[:],
                        op0=mult, op1=mybir.AluOpType.add)
                po = ps_pool.tile([128, nb], f32, tag="ps", name=f"po_{up}")
                nc.tensor.matmul(po[0:1, :], ones_sb[:, 0:1], v[:],
                                 start=True, stop=True)
                orow = ov_pool.tile([1, nb], f32, tag="orow", name=f"orow_{up}")
                nc.vector.tensor_scalar_add(orow[:], po[0:1, :],
                                            bias_sb[0:1, 56:57])
                nc.gpsimd.dma_start(y_d[0:1, tp * nb:(tp + 1) * nb], orow[:])

            ntot = repeat * nt

            def load_xt(u):
                t = u % nt
                xt = xt_pool.tile([128, nb], mdt, tag="xt", name=f"xt_{u}")
                for c in range(4):
                    nc.gpsimd.dma_start(xt[32 * c:32 * c + KI, :],
                                        xT_d[:, t * nb:(t + 1) * nb])
                return xt

            def phase_s1(u, xt):
                """S1 = x@Sw.T + b; evacuates on the DVE so the ACT engine
                stays free for R0/G/Z0."""
                s0 = [s_pool.tile([128, nb], mdt, tag=f"s{k}",
                                  name=f"s_{u}_0_{k}") for k in range(NM)]
                ps1 = x_starts(0, xt, f"ps_s1_{u}", single=True)
                for m in range(NM):
                    nc.vector.tensor_scalar_add(s0[m][:], ps1[m][:],
                                                b_ap(0, m))
                return s0

            def phase_gate(u, xt, gate, tag):
                dest = [act_pool.tile([128, nb], mdt, tag=f"{tag}{m}",
                                      name=f"{tag}_{u}_0_{m}")
                        for m in range(NM)]
                ps = x_starts(gate, xt, f"ps_{tag}0_{u}", single=True)
                for m in range(NM):
                    nc.scalar.activation(dest[m][:], ps[m][:], Tanh,
                                         bias=b_ap(gate, m))
                return dest

            # software-pipelined prologue: tile 0's phase A
            xt = load_xt(0)
            load_big_weights()
            state = {
                "s": phase_s1(0, xt),
                "r": phase_gate(0, xt, 3, "r"),
                "g": phase_gate(0, xt, 2, "g"),
                "z": phase_gate(0, xt, 1, "z"),
                "xt": xt,
            }

            for u in range(ntot):
                t = u % nt
                xt = state["xt"]
                s_cur, r_t, g_t, z_t = (state["s"], state["r"], state["g"],
                                        state["z"])
                xt_n = load_xt(u + 1) if u + 1 < ntot else None

                # previous tile's output row fills the SR0/H0 dep gap
                if pend is not None:
                    emit_out(pend)
                    pend = None

                for i in range(N_LAYERS):
                    if i > 0:
                        # R = tanh(br + Ur x + Wg S) — on H's critical
                        # path, so it goes before Z.
                        r_t = [act_pool.tile([128, nb], mdt, tag=f"r{m}",
                                             name=f"r_{u}_{i}_{m}")
                               for m in range(NM)]
                        wgroup(5, wg_sb, xt, s_cur, r_t, u, i, "r")
                        # Z = tanh(bz + Uz x + Wz S)
                        z_t = [act_pool.tile([128, nb], mdt, tag=f"z{m}",
                                             name=f"z_{u}_{i}_{m}")
                               for m in range(NM)]
                        wgroup(4, wz_sb, xt, s_cur, z_t, u, i, "z")

                    # SR = S * R, in place into R's tiles
                    for k in range(NM):
                        nc.vector.tensor_mul(r_t[k][:], s_cur[k][:],
                                             r_t[k][:])

                    # H = tanh(bh + Uh x + Wg (S*R))
                    h_t = [act_pool.tile([128, nb], mdt, tag=f"h{m}",
                                         name=f"h_{u}_{i}_{m}")
                           for m in range(NM)]
                    # Z*S can run on the DVE during H's matmuls
                    for m in range(NM):
                        nc.vector.tensor_mul(z_t[m][:], z_t[m][:],
                                             s_cur[m][:])
                    wgroup(6, wg_sb, xt, r_t, h_t, u, i, "h")

                    # output = (1-G)*H + Z*S == (Z*S) - (G-1)*H
                    for m in range(NM):
                        nc.vector.scalar_tensor_tensor(
                            h_t[m][:], g_t[m][:], 1.0, h_t[m][:],
                            op0=sub, op1=mult)
                        nc.vector.tensor_sub(h_t[m][:], z_t[m][:],
                                             h_t[m][:])

                    if i < N_LAYERS - 1:
                        s_new = [s_pool.tile([128, nb], mdt, tag=f"s{k}",
                                             name=f"s_{u}_{i + 1}_{k}")
                                 for k in range(NM)]
                        for m in range(NM):
                            nc.scalar.activation(s_new[m][:], h_t[m][:],
                                                 Tanh)
                        s_cur = s_new

                    # next tile's phase-A pieces fill the PE stall where
                    # layer i+1's first k-matmul waits on s_new planes.
                    # (Placement is WAR-constrained: r/z prefetches must come
                    # after tile t's last alloc of the same tag at bufs=2.)
                    if xt_n is not None:
                        if i == 0:
                            state["s"] = phase_s1(u + 1, xt_n)
                        elif i == 1:
                            state["g"] = phase_gate(u + 1, xt_n, 2, "g")
                        elif i == 2:
                            state["r"] = phase_gate(u + 1, xt_n, 3, "r")
                            state["z"] = phase_gate(u + 1, xt_n, 1, "z")

                # y = out_w @ output + out_b, deferred into the next
                # tile's H0 phase
                pend = (h_t, t, u)
                state["xt"] = xt_n

            if pend is not None:
                emit_out(pend)

    nc.compile()
    return nc


def _get_nc(bc=BC, nb=NB, mm_dt=MM_DT):
    key = (bc, nb, mm_dt)
    if key not in _BUILD_CACHE:
        _BUILD_CACHE[key] = _build(bc, nb, mm_dt)
    return _BUILD_CACHE[key]


def _prep_inputs(x, Sw_w, Sw_b, Uz_w, Uz_b, Wz_w, Wz_b, Ug_w, Ug_b, Wg_w,
                 Wg_b, Ur_w, Ur_b, Uh_w, Uh_b, out_w, out_b):
    f = np.float32
    d = np.float64
    xT = np.ascontiguousarray(np.asarray(x, f).T)               # [16, B]
    WzT = np.ascontiguousarray(np.asarray(Wz_w, f).T)           # [H, H]
    WgT = np.ascontiguousarray(np.asarray(Wg_w, f).T)

    # layer-0 folds (S1 is affine in x)
    Swd, Swbd = np.asarray(Sw_w, d), np.asarray(Sw_b, d)
    Wzd, Wgd = np.asarray(Wz_w, d), np.asarray(Wg_w, d)
    MzS = Wzd @ Swd            # [H, 16]
    MgS = Wgd @ Swd
    czS = Wzd @ Swbd           # [H]
    cgS = Wgd @ Swbd

    ublocks = [
        np.asarray(Sw_w, d),
        np.asarray(Uz_w, d) + MzS,
        np.asarray(Ug_w, d) + MgS,
        np.asarray(Ur_w, d) + MgS,
        np.asarray(Uz_w, d),
        np.asarray(Ur_w, d),
        np.asarray(Uh_w, d),
    ]
    U16 = np.concatenate([w.T for w in ublocks], axis=1).astype(f)  # [16,7H]
    U = np.zeros((128, NG * H), f)
    for c in range(4):
        U[32 * c:32 * c + KI] = U16

    combos = [
        np.asarray(Sw_b, d),
        np.asarray(Uz_b, d) + np.asarray(Wz_b, d) + czS,
        np.asarray(Ug_b, d) + np.asarray(Wg_b, d) + cgS,
        np.asarray(Ur_b, d) + np.asarray(Wg_b, d) + cgS,
        np.asarray(Uz_b, d) + np.asarray(Wz_b, d),
        np.asarray(Ur_b, d) + np.asarray(Wg_b, d),
        np.asarray(Uh_b, d) + np.asarray(Wg_b, d),
    ]
    bias = np.zeros((128, 64), f)
    for g, b in enumerate(combos):
        bias[:, g * NM:(g + 1) * NM] = b.astype(f).reshape(NM, 128).T
    bias[:, 56] = np.float32(np.asarray(out_b, f)[0])
    OW = np.ascontiguousarray(np.asarray(out_w, f).reshape(NM, 128).T)
    return xT, WzT, WgT, U, bias, OW


def kernel(**inputs):
    from concourse.bass_utils import run_bass_kernel_spmd

    nc = _get_nc()
    in_maps = _make_in_maps(inputs)
    res = run_bass_kernel_spmd(nc, in_maps, list(range(NCORES)))
    y = np.concatenate([res.results[c]["Y"] for c in range(NCORES)], axis=1)
    return np.ascontiguousarray(y.reshape(B_FULL, 1)).astype(np.float32)


def _make_in_maps(inputs, mm_dt=MM_DT):
    from concourse import mybir

    xT, WzT, WgT, U, bias, OW = _prep_inputs(**inputs)
    mnp = mybir.dt.np(getattr(mybir.dt, mm_dt))
    cast = lambda a: np.ascontiguousarray(a).astype(mnp)
    return [{
        "xT": cast(xT[:, c * BC:(c + 1) * BC]),
        "WzT": cast(WzT), "WgT": cast(WgT), "U": cast(U), "BIAS": bias,
        "OW": OW,
    } for c in range(NCORES)]


def timed_run(inputs, iters=5, nc=None, pipeline=1):
    """Build a persistent jitted runner (so walrus compiles once), stage the
    inputs on-device, and time repeated executions. Returns (best_ns,
    all_ns, output)."""
    import time
    import jax
    from jax.sharding import Mesh, PartitionSpec, NamedSharding
    from jax.experimental.shard_map import shard_map
    from concourse import bass2jax, mybir

    bass2jax.install_neuronx_cc_hook()
    if nc is None:
        nc = _get_nc()
    in_maps = _make_in_maps(inputs)
    n_cores = NCORES

    partition_name = (nc.partition_id_tensor.name
                      if nc.partition_id_tensor else None)
    in_names, out_names, out_avals, zero_outs = [], [], [], []
    for alloc in nc.m.functions[0].allocations:
        if not isinstance(alloc, mybir.MemoryLocationSet):
            continue
        name = alloc.memorylocations[0].name
        if alloc.kind == "ExternalInput":
            if name != partition_name:
                in_names.append(name)
        elif alloc.kind == "ExternalOutput":
            shape = tuple(alloc.tensor_shape)
            dtype = mybir.dt.np(alloc.dtype)
            out_names.append(name)
            out_avals.append(jax.core.ShapedArray(shape, dtype))
            zero_outs.append(np.zeros(shape, dtype))
    n_params = len(in_names)
    n_outs = len(out_avals)
    all_in = list(in_names) + list(out_names)
    if partition_name is not None:
        all_in.append(partition_name)
    donate = tuple(range(n_params, n_params + n_outs))

    def _body(*args):
        operands = list(args)
        if partition_name is not None:
            operands.append(bass2jax.partition_id_tensor())
        outs = bass2jax._bass_exec_p.bind(
            *operands,
            out_avals=tuple(out_avals),
            in_names=tuple(all_in),
            out_names=tuple(out_names),
            lowering_input_output_aliases=(),
            sim_require_finite=True,
            sim_require_nnan=True,
            nc=nc,
        )
        return tuple(outs)

    devices = jax.devices()[:n_cores]
    mesh = Mesh(np.asarray(devices), ("core",))
    spec = PartitionSpec("core")
    sharded = jax.jit(
        shard_map(_body, mesh=mesh, in_specs=(spec,) * (n_params + n_outs),
                  out_specs=(spec,) * n_outs, check_rep=False),
        donate_argnums=donate, keep_unused=True)

    sharding = NamedSharding(mesh, spec)
    dev_in = [
        jax.device_put(
            np.concatenate([np.asarray(in_maps[c][n]) for c in range(n_cores)],
                           axis=0), sharding)
        for n in in_names
    ]
    def fresh_zeros():
        return [np.zeros((n_cores * z.shape[0], *z.shape[1:]), z.dtype)
                for z in zero_outs]

    # warmup (compiles)
    outs = sharded(*dev_in, *fresh_zeros())
    jax.block_until_ready(outs)

    state = {"outs": outs}

    def run_once(pipeline_n=pipeline):
        zss = [fresh_zeros() for _ in range(pipeline_n)]
        t0 = time.perf_counter()
        all_outs = [sharded(*dev_in, *zs) for zs in zss]
        jax.block_until_ready(all_outs)
        state["outs"] = all_outs[-1]
        return int((time.perf_counter() - t0) * 1e9 / pipeline_n)

    def get_y():
        y = np.asarray(state["outs"][out_names.index("Y")])  # [8, BC]
        return np.ascontiguousarray(
            y.reshape(1, B_FULL).reshape(B_FULL, 1)).astype(np.float32)

    if iters is None:
        return run_once, get_y

    times = [run_once() for _ in range(iters)]
    return min(times), times, get_y()


# revision 28
# speedup vs baseline: 1.6456x; 1.4893x over previous
"""DGMNet (dense MLP, 4 DGM layers) Trainium2 kernel.

Strategy: data-parallel over the batch dim (65536 rows -> 8 cores x 8192).
Inside each core, activations live feature-major in SBUF ([128 features x
batch-tile] tiles); every gate matmul is out[M=feat,N=batch] =
W.T-slice @ S with PE accumulation over the 1024-feature contraction, and
the x-side projections (K=16) are folded into the same PSUM accumulation
group, issued 4-wide via row tiling.

Algebraic reduction: S1 = x@Sw.T + b is affine in the 16-dim input, so
every layer-0 term that is linear in S1 collapses into a K=16 matmul with
host-folded weights:
    Wz@S1 -> x@(Wz Sw).T + Wz Sw_b      (layer-0 Z gate)
    Wg@S1 -> x@(Wg Sw).T + Wg Sw_b      (G gate and layer-0 R gate)
This removes 2 of the 12 HxH matmuls per batch element (and the whole
wgS1 tensor + its DVE adds/ACT evacuations).

Matmuls run in bfloat16 (fp8 was measured to blow the 2e-2 error budget:
weight-only e4m3 already gives 2.4e-2). PSUM accumulation stays fp32;
biases ride the ACT evacuation; the elementwise combine runs on the DVE
in bf16 via scalar_tensor_tensor ((G-1)*H) so no (1-G) precompute is
needed. The scalar output row of tile t is emitted during tile t+1's
phase-A so the PE never waits on the layer-3 tail.
"""

import sys

sys.path.insert(0, "/opt/trn_rl_repo")

import numpy as np

B_FULL = 65536
KI = 16
H = 1024
NCORES = 8
BC = B_FULL // NCORES  # per-core batch (8192)
NB = 512               # batch tile (one PSUM bank of fp32)
NM = H // 128          # feature tiles (8)
N_LAYERS = 4
NG = 7                 # U gate blocks: S1, Z0', G', R0', Z, R, H

MM_DT = "bfloat16"
# Layers 1-3 R and Z gates run their HxH matmuls in fp8e4 with DoubleRow
# (2 contraction chunks per pass). Error budget: measured 1.42e-2 on the
# harness inputs vs the 2e-2 gate (bf16-everywhere is 6.1e-3). The H gate
# stays bf16 — fp8 there pushes past the gate (1.9e-2 alone).
FP8_RZ = True

_BUILD_CACHE = {}


def _build(bc, nb, mm_dt=MM_DT, repeat=1, fp8_rz=None):
    """Build + compile the single-core Bass program. Returns nc.

    repeat > 1 re-runs the whole computation (for slope-based timing of the
    device execution under the large axon dispatch overhead)."""
    import concourse.bacc as bacc
    import concourse.mybir as mybir
    import concourse.tile as tile

    if fp8_rz is None:
        fp8_rz = FP8_RZ
    f32 = mybir.dt.float32
    mdt = getattr(mybir.dt, mm_dt)
    f8 = mybir.dt.float8e4
    DR = mybir.MatmulPerfMode.DoubleRow
    Tanh = mybir.ActivationFunctionType.Tanh
    mult = mybir.AluOpType.mult
    sub = mybir.AluOpType.subtract

    nt = bc // nb

    nc = bacc.Bacc("TRN2", target_bir_lowering=False, debug=False,
                   num_devices=NCORES)

    xT_d = nc.dram_tensor("xT", [KI, bc], mdt, kind="ExternalInput").ap()
    wg_d = nc.dram_tensor("WgT", [H, H], mdt, kind="ExternalInput").ap()
    if fp8_rz:
        wz8_d = nc.dram_tensor("WzDR", [128, NM * H], f8,
                               kind="ExternalInput").ap()
        wg8_d = nc.dram_tensor("WgDR", [128, NM * H], f8,
                               kind="ExternalInput").ap()
    else:
        wz_d = nc.dram_tensor("WzT", [H, H], mdt, kind="ExternalInput").ap()
    u_d = nc.dram_tensor("U", [128, NG * H], mdt, kind="ExternalInput").ap()
    bias_d = nc.dram_tensor("BIAS", [128, 64], f32, kind="ExternalInput").ap()
    ow_d = nc.dram_tensor("OW", [128, NM], f32, kind="ExternalInput").ap()
    y_d = nc.dram_tensor("Y", [1, bc], f32, kind="ExternalOutput").ap()

    with tile.TileContext(nc) as tc:
        with (
            tc.tile_pool(name="const", bufs=1) as cpool,
            tc.tile_pool(name="xt", bufs=3) as xt_pool,
            tc.tile_pool(name="s", bufs=3) as s_pool,
            tc.tile_pool(name="s8", bufs=2) as s8_pool,
            tc.tile_pool(name="act", bufs=2) as act_pool,
            tc.tile_pool(name="ov", bufs=2) as ov_pool,
            tc.tile_pool(name="psum", bufs=8, space="PSUM") as ps_pool,
        ):
            # ---- resident constants. Order matters: the first tile's
            # phase A needs only U + bias + x, so those go ahead of the
            # 4 MB of Wg/Wz; Wg is needed a layer-phase before Wz. ---------
            u_sb = cpool.tile([128, NG * H], mdt)
            nc.gpsimd.dma_start(u_sb[:], u_d[:])
            bias_sb = cpool.tile([128, 64], f32)
            nc.gpsimd.dma_start(bias_sb[:], bias_d[:])
            wg_sb = cpool.tile([128, NM * H], mdt)
            ow_sb = cpool.tile([128, NM], f32)
            ones_sb = cpool.tile([128, 1], mdt)
            nc.gpsimd.memset(ones_sb[:], 1.0)
            if fp8_rz:
                wg8_sb = cpool.tile([128, NM, H], f8)
                wz8_sb = cpool.tile([128, NM, H], f8)
            else:
                wz_sb = cpool.tile([128, NM * H], mdt)

            def load_big_weights():
                for k in range(NM):
                    nc.gpsimd.dma_start(wg_sb[:, k * H:(k + 1) * H],
                                        wg_d[k * 128:(k + 1) * 128, :])
                nc.gpsimd.dma_start(ow_sb[:], ow_d[:])
                if fp8_rz:
                    for k in range(NM):
                        nc.gpsimd.dma_start(wg8_sb[:, k, :],
                                            wg8_d[:, k * H:(k + 1) * H])
                    for k in range(NM):
                        nc.gpsimd.dma_start(wz8_sb[:, k, :],
                                            wz8_d[:, k * H:(k + 1) * H])
                else:
                    for k in range(NM):
                        nc.gpsimd.dma_start(wz_sb[:, k * H:(k + 1) * H],
                                            wz_d[k * 128:(k + 1) * 128, :])

            def w_ap(w_sb, k, m):
                return w_sb[:, k * H + m * 128:k * H + (m + 1) * 128]

            def u_ap(g, m, c):
                return u_sb[32 * c:32 * c + KI,
                            g * H + m * 128:g * H + (m + 1) * 128]

            def b_ap(g, m):
                return bias_sb[:, g * NM + m:g * NM + m + 1]

            def x_starts(gate, xt, nametag, single):
                """Row-tiled (4-concurrent) K=16 start matmuls for m-quads."""
                pss = [None] * NM
                for mq in (0, 4):
                    for c in range(4):
                        m = mq + c
                        pss[m] = ps_pool.tile([128, nb], f32, tag="ps",
                                              name=f"{nametag}_{m}")
                    for c in range(4):
                        m = mq + c
                        nc.tensor.matmul(
                            pss[m][:], u_ap(gate, m, c),
                            xt[32 * c:32 * c + KI, :],
                            start=True, stop=single,
                            tile_position=(32 * c, 0))
                return pss

            def wgroup(gate, w_sb, xt, rhs, dest, t_u, li, tag):
                """Full gate: U-start quad + 8 K-chunk matmuls per m, tanh
                evacuation with bias into dest[m]."""
                for mq in (0, 4):
                    pss = {}
                    for c in range(4):
                        m = mq + c
                        pss[m] = ps_pool.tile([128, nb], f32, tag="ps",
                                              name=f"ps_{tag}_{t_u}_{li}_{m}")
                    for c in range(4):
                        m = mq + c
                        nc.tensor.matmul(pss[m][:], u_ap(gate, m, c),
                                         xt[32 * c:32 * c + KI, :],
                                         start=True, stop=False,
                                         tile_position=(32 * c, 0))
                    for c in range(4):
                        m = mq + c
                        for k in range(NM):
                            nc.tensor.matmul(pss[m][:], w_ap(w_sb, k, m),
                                             rhs[k][:],
                                             start=False, stop=(k == NM - 1))
                        nc.scalar.activation(dest[m][:], pss[m][:], Tanh,
                                             bias=b_ap(gate, m))

            def wgroup_dr(gate, w8_sb, xt, s8, dest, t_u, li, tag):
                """fp8 DoubleRow gate: U-start quad (bf16) + 4 double-chunk
                matmuls per m (each contracts 256 features)."""
                for mq in (0, 4):
                    pss = {}
                    for c in range(4):
                        m = mq + c
                        pss[m] = ps_pool.tile([128, nb], f32, tag="ps",
                                              name=f"ps_{tag}_{t_u}_{li}_{m}")
                    for c in range(4):
                        m = mq + c
                        nc.tensor.matmul(pss[m][:], u_ap(gate, m, c),
                                         xt[32 * c:32 * c + KI, :],
                                         start=True, stop=False,
                                         tile_position=(32 * c, 0))
                    for c in range(4):
                        m = mq + c
                        for kp in range(NM // 2):
                            nc.tensor.matmul(
                                pss[m][:],
                                w8_sb[:, 2 * kp:2 * kp + 2,
                                      m * 128:(m + 1) * 128],
                                s8[:, 2 * kp:2 * kp + 2, :],
                                start=False, stop=(kp == NM // 2 - 1),
                                perf_mode=DR)
                        nc.scalar.activation(dest[m][:], pss[m][:], Tanh,
                                             bias=b_ap(gate, m))

            # ---- per batch tile -----------------------------------------
            pend = None  # deferred output row of the previous batch tile

            def emit_out(pend):
                # y-row = out_w @ h: per-partition weighted accumulation on
                # the DVE (8 fused mul-adds), then a single ones-matmul for
                # the partition reduction — 1 PE matmul instead of 8.
                h_prev, tp, up = pend
                v = ov_pool.tile([128, nb], mdt, tag="vrow", name=f"v_{up}")
                nc.vector.tensor_scalar_mul(v[:], h_prev[0][:],
                                            ow_sb[:, 0:1])
                for k in range(1, NM):
                    nc.vector.scalar_tensor_tensor(
                        v[:], h_prev[k][:], ow_sb[:, k:k + 1], v[:],
                        op0=mult, op1=mybir.AluOpType.add)
                po = ps_pool.tile([128, nb], f32, tag="ps", name=f"po_{up}")
                nc.tensor.matmul(po[0:1, :], ones_sb[:, 0:1], v[:],
                                 start=True, stop=True)
                orow = ov_pool.tile([1, nb], f32, tag="orow", name=f"orow_{up}")
                nc.vector.tensor_scalar_add(orow[:], po[0:1, :],
                                            bias_sb[0:1, 56:57])
                nc.gpsimd.dma_start(y_d[0:1, tp * nb:(tp + 1) * nb], orow[:])

            ntot = repeat * nt

            def load_xt(u):
                t = u % nt
                xt = xt_pool.tile([128, nb], mdt, tag="xt", name=f"xt_{u}")
                for c in range(4):
                    nc.gpsimd.dma_start(xt[32 * c:32 * c + KI, :],
                                        xT_d[:, t * nb:(t + 1) * nb])
                return xt

            def phase_s1(u, xt):
                """S1 = x@Sw.T + b; evacuates on the DVE so the ACT engine
                stays free for R0/G/Z0."""
                s0 = [s_pool.tile([128, nb], mdt, tag=f"s{k}",
                                  name=f"s_{u}_0_{k}") for k in range(NM)]
                ps1 = x_starts(0, xt, f"ps_s1_{u}", single=True)
                for m in range(NM):
                    nc.vector.tensor_scalar_add(s0[m][:], ps1[m][:],
                                                b_ap(0, m))
                return s0

            def phase_gate(u, xt, gate, tag):
                dest = [act_pool.tile([128, nb], mdt, tag=f"{tag}{m}",
                                      name=f"{tag}_{u}_0_{m}")
                        for m in range(NM)]
                ps = x_starts(gate, xt, f"ps_{tag}0_{u}", single=True)
                for m in range(NM):
                    nc.scalar.activation(dest[m][:], ps[m][:], Tanh,
                                         bias=b_ap(gate, m))
                return dest

            # software-pipelined prologue: tile 0's phase A
            xt = load_xt(0)
            load_big_weights()
            state = {
                "s": phase_s1(0, xt),
                "r": phase_gate(0, xt, 3, "r"),
                "g": phase_gate(0, xt, 2, "g"),
                "z": phase_gate(0, xt, 1, "z"),
                "xt": xt,
            }

            for u in range(ntot):
                t = u % nt
                xt = state["xt"]
                s_cur, r_t, g_t, z_t = (state["s"], state["r"], state["g"],
                                        state["z"])
                xt_n = load_xt(u + 1) if u + 1 < ntot else None

                # previous tile's output row fills the SR0/H0 dep gap
                if pend is not None:
                    emit_out(pend)
                    pend = None

                s8 = None
                for i in range(N_LAYERS):
                    if i > 0:
                        # R = tanh(br + Ur x + Wg S) — on H's critical
                        # path, so it goes before Z.
                        r_t = [act_pool.tile([128, nb], mdt, tag=f"r{m}",
                                             name=f"r_{u}_{i}_{m}")
                               for m in range(NM)]
                        z_t = [act_pool.tile([128, nb], mdt, tag=f"z{m}",
                                             name=f"z_{u}_{i}_{m}")
                               for m in range(NM)]
                        if fp8_rz:
                            wgroup_dr(5, wg8_sb, xt, s8, r_t, u, i, "r")
                            wgroup_dr(4, wz8_sb, xt, s8, z_t, u, i, "z")
                        else:
                            wgroup(5, wg_sb, xt, s_cur, r_t, u, i, "r")
                            wgroup(4, wz_sb, xt, s_cur, z_t, u, i, "z")

                    # SR = S * R, in place into R's tiles
                    for k in range(NM):
                        nc.vector.tensor_mul(r_t[k][:], s_cur[k][:],
                                             r_t[k][:])

                    # H = tanh(bh + Uh x + Wg (S*R))
                    h_t = [act_pool.tile([128, nb], mdt, tag=f"h{m}",
                                         name=f"h_{u}_{i}_{m}")
                           for m in range(NM)]
                    # Z*S can run on the DVE during H's matmuls
                    for m in range(NM):
                        nc.vector.tensor_mul(z_t[m][:], z_t[m][:],
                                             s_cur[m][:])
                    wgroup(6, wg_sb, xt, r_t, h_t, u, i, "h")

                    # output = (1-G)*H + Z*S == (Z*S) - (G-1)*H
                    for m in range(NM):
                        nc.vector.scalar_tensor_tensor(
                            h_t[m][:], g_t[m][:], 1.0, h_t[m][:],
                            op0=sub, op1=mult)
                        nc.vector.tensor_sub(h_t[m][:], z_t[m][:],
                                             h_t[m][:])

                    if i < N_LAYERS - 1:
                        s_new = [s_pool.tile([128, nb], mdt, tag=f"s{k}",
                                             name=f"s_{u}_{i + 1}_{k}")
                                 for k in range(NM)]
                        for m in range(NM):
                            nc.scalar.activation(s_new[m][:], h_t[m][:],
                                                 Tanh)
                        if fp8_rz:
                            # fp8 copy of S for the next layer's R/Z
                            # DoubleRow matmuls (bf16 copy stays for the
                            # SR product and the Z*S combine term)
                            s8 = s8_pool.tile([128, NM, nb], f8, tag="s8",
                                              name=f"s8_{u}_{i + 1}")
                            for m in range(NM):
                                nc.vector.tensor_copy(s8[:, m, :],
                                                      s_new[m][:])
                        s_cur = s_new

                    # next tile's phase-A pieces fill the PE stall where
                    # layer i+1's first k-matmul waits on s_new planes.
                    # (Placement is WAR-constrained: r/z prefetches must come
                    # after tile t's last alloc of the same tag at bufs=2.)
                    if xt_n is not None:
                        if i == 0:
                            state["s"] = phase_s1(u + 1, xt_n)
                        elif i == 1:
                            state["g"] = phase_gate(u + 1, xt_n, 2, "g")
                        elif i == 2:
                            state["r"] = phase_gate(u + 1, xt_n, 3, "r")
                            state["z"] = phase_gate(u + 1, xt_n, 1, "z")

                # y = out_w @ output + out_b, deferred into the next
                # tile's H0 phase
                pend = (h_t, t, u)
                state["xt"] = xt_n

            if pend is not None:
                emit_out(pend)

    nc.compile()
    return nc


def _get_nc(bc=BC, nb=NB, mm_dt=MM_DT):
    key = (bc, nb, mm_dt)
    if key not in _BUILD_CACHE:
        _BUILD_CACHE[key] = _build(bc, nb, mm_dt)
    return _BUILD_CACHE[key]


def _prep_inputs(x, Sw_w, Sw_b, Uz_w, Uz_b, Wz_w, Wz_b, Ug_w, Ug_b, Wg_w,
                 Wg_b, Ur_w, Ur_b, Uh_w, Uh_b, out_w, out_b):
    f = np.float32
    d = np.float64
    xT = np.ascontiguousarray(np.asarray(x, f).T)               # [16, B]
    WzT = np.ascontiguousarray(np.asarray(Wz_w, f).T)           # [H, H]
    WgT = np.ascontiguousarray(np.asarray(Wg_w, f).T)

    # layer-0 folds (S1 is affine in x)
    Swd, Swbd = np.asarray(Sw_w, d), np.asarray(Sw_b, d)
    Wzd, Wgd = np.asarray(Wz_w, d), np.asarray(Wg_w, d)
    MzS = Wzd @ Swd            # [H, 16]
    MgS = Wgd @ Swd
    czS = Wzd @ Swbd           # [H]
    cgS = Wgd @ Swbd

    ublocks = [
        np.asarray(Sw_w, d),
        np.asarray(Uz_w, d) + MzS,
        np.asarray(Ug_w, d) + MgS,
        np.asarray(Ur_w, d) + MgS,
        np.asarray(Uz_w, d),
        np.asarray(Ur_w, d),
        np.asarray(Uh_w, d),
    ]
    U16 = np.concatenate([w.T for w in ublocks], axis=1).astype(f)  # [16,7H]
    U = np.zeros((128, NG * H), f)
    for c in range(4):
        U[32 * c:32 * c + KI] = U16

    combos = [
        np.asarray(Sw_b, d),
        np.asarray(Uz_b, d) + np.asarray(Wz_b, d) + czS,
        np.asarray(Ug_b, d) + np.asarray(Wg_b, d) + cgS,
        np.asarray(Ur_b, d) + np.asarray(Wg_b, d) + cgS,
        np.asarray(Uz_b, d) + np.asarray(Wz_b, d),
        np.asarray(Ur_b, d) + np.asarray(Wg_b, d),
        np.asarray(Uh_b, d) + np.asarray(Wg_b, d),
    ]
    bias = np.zeros((128, 64), f)
    for g, b in enumerate(combos):
        bias[:, g * NM:(g + 1) * NM] = b.astype(f).reshape(NM, 128).T
    bias[:, 56] = np.float32(np.asarray(out_b, f)[0])
    OW = np.ascontiguousarray(np.asarray(out_w, f).reshape(NM, 128).T)
    # DoubleRow fp8 layout: wDR[p, k*H + col] = W.T[128k + p, col]
    dr = lambda WT: np.ascontiguousarray(
        WT.reshape(NM, 128, H).transpose(1, 0, 2).reshape(128, NM * H))
    return xT, WzT, WgT, U, bias, OW, dr(WzT), dr(WgT)


def kernel(**inputs):
    from concourse.bass_utils import run_bass_kernel_spmd

    nc = _get_nc()
    in_maps = _make_in_maps(inputs)
    res = run_bass_kernel_spmd(nc, in_maps, list(range(NCORES)))
    y = np.concatenate([res.results[c]["Y"] for c in range(NCORES)], axis=1)
    return np.ascontiguousarray(y.reshape(B_FULL, 1)).astype(np.float32)


def _make_in_maps(inputs, mm_dt=MM_DT, fp8_rz=None):
    from concourse import mybir

    if fp8_rz is None:
        fp8_rz = FP8_RZ
    xT, WzT, WgT, U, bias, OW, WzDR, WgDR = _prep_inputs(**inputs)
    mnp = mybir.dt.np(getattr(mybir.dt, mm_dt))
    f8np = mybir.dt.np(mybir.dt.float8e4)
    cast = lambda a: np.ascontiguousarray(a).astype(mnp)
    base = {
        "WgT": cast(WgT), "U": cast(U), "BIAS": bias, "OW": OW,
    }
    if fp8_rz:
        base["WzDR"] = WzDR.astype(f8np)
        base["WgDR"] = WgDR.astype(f8np)
    else:
        base["WzT"] = cast(WzT)
    return [{
        "xT": cast(xT[:, c * BC:(c + 1) * BC]), **base,
    } for c in range(NCORES)]


def timed_run(inputs, iters=5, nc=None, pipeline=1):
    """Build a persistent jitted runner (so walrus compiles once), stage the
    inputs on-device, and time repeated executions. Returns (best_ns,
    all_ns, output)."""
    import time
    import jax
    from jax.sharding import Mesh, PartitionSpec, NamedSharding
    from jax.experimental.shard_map import shard_map
    from concourse import bass2jax, mybir

    bass2jax.install_neuronx_cc_hook()
    if nc is None:
        nc = _get_nc()
    in_maps = _make_in_maps(inputs)
    n_cores = NCORES

    partition_name = (nc.partition_id_tensor.name
                      if nc.partition_id_tensor else None)
    in_names, out_names, out_avals, zero_outs = [], [], [], []
    for alloc in nc.m.functions[0].allocations:
        if not isinstance(alloc, mybir.MemoryLocationSet):
            continue
        name = alloc.memorylocations[0].name
        if alloc.kind == "ExternalInput":
            if name != partition_name:
                in_names.append(name)
        elif alloc.kind == "ExternalOutput":
            shape = tuple(alloc.tensor_shape)
            dtype = mybir.dt.np(alloc.dtype)
            out_names.append(name)
            out_avals.append(jax.core.ShapedArray(shape, dtype))
            zero_outs.append(np.zeros(shape, dtype))
    n_params = len(in_names)
    n_outs = len(out_avals)
    all_in = list(in_names) + list(out_names)
    if partition_name is not None:
        all_in.append(partition_name)
    donate = tuple(range(n_params, n_params + n_outs))

    def _body(*args):
        operands = list(args)
        if partition_name is not None:
            operands.append(bass2jax.partition_id_tensor())
        outs = bass2jax._bass_exec_p.bind(
            *operands,
            out_avals=tuple(out_avals),
            in_names=tuple(all_in),
            out_names=tuple(out_names),
            lowering_input_output_aliases=(),
            sim_require_finite=True,
            sim_require_nnan=True,
            nc=nc,
        )
        return tuple(outs)

    devices = jax.devices()[:n_cores]
    mesh = Mesh(np.asarray(devices), ("core",))
    spec = PartitionSpec("core")
    sharded = jax.jit(
        shard_map(_body, mesh=mesh, in_specs=(spec,) * (n_params + n_outs),
                  out_specs=(spec,) * n_outs, check_rep=False),
        donate_argnums=donate, keep_unused=True)

    sharding = NamedSharding(mesh, spec)
    dev_in = [
        jax.device_put(
            np.concatenate([np.asarray(in_maps[c][n]) for c in range(n_cores)],
                           axis=0), sharding)
        for n in in_names
    ]
    def fresh_zeros():
        return [np.zeros((n_cores * z.shape[0], *z.shape[1:]), z.dtype)
                for z in zero_outs]

    # warmup (compiles)
    outs = sharded(*dev_in, *fresh_zeros())
    jax.block_until_ready(outs)

    state = {"outs": outs}

    def run_once(pipeline_n=pipeline):
        zss = [fresh_zeros() for _ in range(pipeline_n)]
        t0 = time.perf_counter()
        all_outs = [sharded(*dev_in, *zs) for zs in zss]
        jax.block_until_ready(all_outs)
        state["outs"] = all_outs[-1]
        return int((time.perf_counter() - t0) * 1e9 / pipeline_n)

    def get_y():
        y = np.asarray(state["outs"][out_names.index("Y")])  # [8, BC]
        return np.ascontiguousarray(
            y.reshape(1, B_FULL).reshape(B_FULL, 1)).astype(np.float32)

    if iters is None:
        return run_once, get_y

    times = [run_once() for _ in range(iters)]
    return min(times), times, get_y()
